# revision 18
# baseline (speedup 1.0000x reference)
"""Masked-attention kernel for 8 TRN2 NeuronCores (batch-parallel sharding).

v2 layout strategy: all transposes are done on the HOST (numpy) so the
device touches data only in matmul-native layouts:
  - Q, K are pre-transposed on host to [B, D, S]; DMA lands them directly
    as [d=partition, s] tiles (contiguous per-partition runs), DVE casts
    fp32->fp16. No PE transposes, no staging copies.
  - mask is pre-transposed on host to [B, S_k, S_q] u8; DMA casts u8->fp8
    into [k=partition, q] tiles. Mask folds into the scores inside the PE
    accumulation as 2 N=512 fp8 matmuls per k-tile: stationary -240*I,
    moving mask^T chunk; exp() then flushes masked entries to ~0.
  - scores are computed transposed (S^T[k, q]) so the PV matmul consumes
    exp() output directly with V in its natural [k, d] layout.
  - softmax denominator: DVE accumulates exp tiles across k-tiles; one
    all-ones [128,128] stationary matmul broadcasts den[q] to every PSUM
    partition; DVE reciprocal + one tensor_tensor multiply scales the
    accumulated PV output.
  - output is stored transposed [B, D, S] and un-transposed on host.
"""

import numpy as np
import ml_dtypes

B, S, D = 16, 2048, 128
NCORES = 8
BP = B // NCORES  # batches per core
P = 128
QC = 1024  # q-chunk (columns of the transposed score tile)
NQC = S // QC
NKT = S // P  # k tiles
MM_N = 512  # matmul moving free dim (one PSUM bank of fp32)
SCALE = 1.0 / float(np.sqrt(128.0))
MASK_NEG = -240.0

_CACHE = {}

# dev-only ablation switches (see ablate.py); empty for the graded path
ABLATE = {}


def build_nc(loop=True):
    import concourse.mybir as mybir
    import concourse.tile as tile
    from concourse import bacc

    fp16 = mybir.dt.float16
    fp32 = mybir.dt.float32

    nc = bacc.Bacc("TRN2", target_bir_lowering=False, debug=False,
                   num_devices=NCORES)

    Qtd = nc.dram_tensor("Qt", [BP, D, S], fp16, kind="ExternalInput")
    Ktd = nc.dram_tensor("Kt", [BP, D, S], fp16, kind="ExternalInput")
    Vd = nc.dram_tensor("V", [BP, S, D], fp16, kind="ExternalInput")
    Md = nc.dram_tensor("maskT", [BP, S, S], mybir.dt.float8e4,
                        kind="ExternalInput")
    if loop:
        # run-count knob for differential HW timing (graded path: loop=False)
        Id = nc.dram_tensor("iters", [1, 1], mybir.dt.int32,
                            kind="ExternalInput")
    Od = nc.dram_tensor("outT", [BP, D, S], fp32, kind="ExternalOutput")

    negI_np = (MASK_NEG * np.eye(P, dtype=np.float32)).astype(
        ml_dtypes.float8_e4m3)
    negI_dram = nc.inline_tensor(negI_np, name="negI_const")
    ones_dram = nc.inline_tensor(np.ones((P, P), dtype=np.float16),
                                 name="ones_const")

    with tile.TileContext(nc) as tc:
        with tc.tile_pool(name="consts", bufs=1) as consts, \
             tc.tile_pool(name="stag", bufs=2) as stag, \
             tc.tile_pool(name="qkv", bufs=1) as qkv, \
             tc.tile_pool(name="maskp", bufs=6) as maskp, \
             tc.tile_pool(name="pp", bufs=4) as pp, \
             tc.tile_pool(name="accp", bufs=2) as accp, \
             tc.tile_pool(name="outp", bufs=2) as outp, \
             tc.tile_pool(name="spsum", bufs=3, space="PSUM") as spsum, \
             tc.tile_pool(name="opsum", bufs=1, space="PSUM") as opsum:

            negI = consts.tile([P, P], mybir.dt.float8e4)
            nc.sync.dma_start(out=negI[:, :], in_=negI_dram.ap())
            ones_mat = consts.tile([P, P], fp16)
            nc.sync.dma_start(out=ones_mat[:, :], in_=ones_dram.ap())

            pools = (stag, qkv, maskp, pp, accp, outp, spsum, opsum)
            if loop:
                it_sb = consts.tile([1, 1], mybir.dt.int32)
                nc.sync.dma_start(out=it_sb[:, :], in_=Id.ap())
                n_iters = nc.values_load(it_sb[:, :],
                                         skip_runtime_bounds_check=True)
                with tc.For_i(0, n_iters, 1,
                              hint_engines=(mybir.EngineType.PE,
                                            mybir.EngineType.Activation,
                                            mybir.EngineType.DVE,
                                            mybir.EngineType.SP,
                                            mybir.EngineType.Pool)):
                    _kernel_body(nc, mybir, Qtd, Ktd, Vd, Md, Od, negI,
                                 ones_mat, *pools)
            else:
                _kernel_body(nc, mybir, Qtd, Ktd, Vd, Md, Od, negI,
                             ones_mat, *pools)
    nc.compile()
    return nc


def _kernel_body(nc, mybir, Qtd, Ktd, Vd, Md, Od, negI, ones_mat,
                 stag, qkv, maskp, pp, accp, outp, spsum, opsum):
    fp16 = mybir.dt.float16
    fp32 = mybir.dt.float32
    fp8 = mybir.dt.float8e4
    Exp = mybir.ActivationFunctionType.Exp

    HS = S // 2  # half of the s dimension, for chunked loads
    MKT = 4  # k-tiles per mask DMA
    AB = ABLATE

    def load_mask_pair(b, qc, mt):
        # [k=partition, kt-group, q] fp8 (host pre-encoded; plain byte
        # copy so it can ride the HWDGE rings, alternating)
        t = maskp.tile([P, MKT, QC], fp8, name="mfT")
        ring = nc.sync.dma_start if mt % 2 == 0 else nc.scalar.dma_start
        ring(out=t[:, :, :],
             in_=Md.ap()[b, mt * MKT * P:(mt + 1) * MKT * P,
                         qc * QC:(qc + 1) * QC]
                 .rearrange("(t p) q -> p t q", p=P))
        return t

    def prep_batch(b, mf0):
        # Q^T/K^T land directly as [d, s] fp16 (host pre-transposed and
        # pre-cast); V natural fp16. Halves on the two HWDGE rings.
        ktt = qkv.tile([P, S], fp16, name=f"ktt{b}")
        qt = qkv.tile([P, S], fp16, name=f"qt{b}")
        vsb = qkv.tile([P, NKT, D], fp16, name=f"vsb{b}")

        def half(dst, src_ap, h, ring, vshape=False):
            if vshape:
                ring(out=dst[:, h * (NKT // 2):(h + 1) * (NKT // 2), :],
                     in_=src_ap[b, h * HS:(h + 1) * HS, :]
                         .rearrange("(t p) d -> p t d", p=P))
            else:
                ring(out=dst[:, h * HS:(h + 1) * HS],
                     in_=src_ap[b, :, h * HS:(h + 1) * HS])

        half(ktt, Ktd.ap(), 0, nc.sync.dma_start)
        half(qt, Qtd.ap(), 0, nc.scalar.dma_start)
        if mf0 is not None:
            mf0.append(load_mask_pair(b, 0, 0))
        half(vsb, Vd.ap(), 0, nc.sync.dma_start, vshape=True)
        half(ktt, Ktd.ap(), 1, nc.sync.dma_start)
        half(qt, Qtd.ap(), 1, nc.scalar.dma_start)
        half(vsb, Vd.ap(), 1, nc.sync.dma_start, vshape=True)
        return qt, ktt, vsb

    const_pt = None
    if AB.get("pv_const_pt"):
        const_pt = qkv.tile([P, QC], fp16, name="constpt")
        nc.vector.memset(const_pt, 0.001)

    mf00 = []
    prepped = {0: prep_batch(0, mf00)}

    # ---- main flash loop over (batch, q-chunk, k-tile) ----
    for b in range(BP):
        for qc in range(NQC):
            if (b, qc) == (0, 1) and BP > 1:
                prepped[1] = prep_batch(1, None)
            qt, ktt, vsb = prepped[b]
            if AB.get("no_mask_dma"):
                mf = [mf00[0]] * (NKT // MKT)
            elif b == 0 and qc == 0:
                mf = mf00 + [load_mask_pair(b, qc, mt)
                             for mt in range(1, NKT // MKT)]
            else:
                mf = [load_mask_pair(b, qc, mt)
                      for mt in range(NKT // MKT)]
            acc = accp.tile([P, QC], fp16, name="acc")
            if AB.get("no_acc") or AB.get("no_exp"):
                nc.vector.memset(acc, 1.0)
            ops = opsum.tile([P, QC], fp32, name="opsum")
            LAG = 2  # PV trails exp by 2 k-tiles: exp(kt) gets ~2 PE-iters
            pts = {}
            for kt in range(NKT):
                sc = spsum.tile([P, QC], fp32, name="scores")
                mfck = mf[kt // MKT]
                first_mm = AB.get("no_mask_mm", False)
                if not first_mm:
                    for n in range(0, QC, MM_N):
                        # start=True clears the 512-wide PSUM bank; the mask
                        # matmul leads each bank's accumulation group
                        nc.tensor.matmul(
                            sc[:, n:n + MM_N],
                            lhsT=negI[:, :],
                            rhs=mfck[:, kt % MKT, n:n + MM_N],
                            start=True, stop=AB.get("no_qk", False),
                            skip_group_check=True)
                if not AB.get("no_qk"):
                    for n in range(0, QC, MM_N):
                        nc.tensor.matmul(
                            sc[:, n:n + MM_N],
                            lhsT=ktt[:, kt * P:(kt + 1) * P],
                            rhs=qt[:, qc * QC + n:qc * QC + n + MM_N],
                            start=first_mm, stop=True,
                            skip_group_check=True)
                pt = pp.tile([P, QC], fp16, name="pt")
                if not AB.get("no_exp"):
                    nc.scalar.activation(out=pt[:, :], in_=sc[:, :],
                                         func=Exp, scale=SCALE)
                elif const_pt is None:
                    nc.vector.tensor_copy(out=pt[:, :], in_=sc[:, :])
                if AB.get("no_acc") or AB.get("no_exp"):
                    pass
                elif kt == 0:
                    nc.vector.tensor_copy(out=acc[:, :], in_=pt[:, :])
                else:
                    nc.vector.tensor_add(out=acc[:, :], in0=acc[:, :],
                                         in1=pt[:, :])
                pts[kt] = const_pt if const_pt is not None else pt
                if kt >= LAG and not AB.get("no_pv"):
                    j = kt - LAG
                    for n in range(0, QC, MM_N):
                        nc.tensor.matmul(
                            ops[:, n:n + MM_N],
                            lhsT=vsb[:, j, :],
                            rhs=pts[j][:, n:n + MM_N],
                            start=(j == 0), stop=False,
                            skip_group_check=True)
            if not AB.get("no_pv"):
                for j in range(NKT - LAG, NKT):
                    for n in range(0, QC, MM_N):
                        nc.tensor.matmul(
                            ops[:, n:n + MM_N],
                            lhsT=vsb[:, j, :],
                            rhs=pts[j][:, n:n + MM_N],
                            start=False, stop=(j == NKT - 1),
                            skip_group_check=True)

            # denominator broadcast to all partitions in one matmul:
            # den[p, q] = sum_k acc[k, q] for every p (all-ones stationary)
            den = spsum.tile([P, QC], fp32, name="scores")
            for n in range(0, QC, MM_N):
                nc.tensor.matmul(den[:, n:n + MM_N],
                                 lhsT=ones_mat[:, :],
                                 rhs=acc[:, n:n + MM_N],
                                 start=True, stop=True,
                                 skip_group_check=True)
            rcp = outp.tile([P, QC], fp32, name="rcp")
            nc.vector.reciprocal(out=rcp[:, :], in_=den[:, :])
            osf = outp.tile([P, QC], fp32, name="osf")
            nc.vector.tensor_mul(out=osf[:, :], in0=ops[:, :],
                                 in1=rcp[:, :])
            ring = nc.scalar.dma_start if qc % 2 == 0 else nc.sync.dma_start
            ring(out=Od.ap()[b, :, qc * QC:(qc + 1) * QC], in_=osf[:, :])


def _get_nc(loop=False):
    key = f"nc_loop{loop}"
    if key not in _CACHE:
        _CACHE[key] = build_nc(loop=loop)
    return _CACHE[key]


def host_prep(Q, K, V, mask):
    """Slice per core, pre-transpose Q/K/mask, pre-cast to device dtypes.

    The mask is encoded as raw fp8e4m3 bytes (0x38 = 1.0) so the device
    DMA is a plain byte copy on the HWDGE rings (no SWDGE cast needed)."""
    Q = np.asarray(Q, dtype=np.float32)
    K = np.asarray(K, dtype=np.float32)
    V16 = np.asarray(V, dtype=np.float32).astype(np.float16)
    mask_u8 = np.asarray(mask).astype(np.uint8)
    Qt = np.ascontiguousarray(
        Q.transpose(0, 2, 1)).astype(np.float16)
    Kt = np.ascontiguousarray(
        K.transpose(0, 2, 1)).astype(np.float16)
    maskT = np.ascontiguousarray(
        (mask_u8.transpose(0, 2, 1) * np.uint8(0x38)))
    maskT = maskT.view(ml_dtypes.float8_e4m3)
    in_maps = []
    for c in range(NCORES):
        sl = slice(c * BP, (c + 1) * BP)
        in_maps.append({
            "Qt": np.ascontiguousarray(Qt[sl]),
            "Kt": np.ascontiguousarray(Kt[sl]),
            "V": np.ascontiguousarray(V16[sl]),
            "maskT": np.ascontiguousarray(maskT[sl]),
        })
    return in_maps


def gather_out(results):
    """Concat per-core transposed outputs and un-transpose to [B, S, D]."""
    outT = np.concatenate([r["outT"] for r in results], axis=0)
    return np.ascontiguousarray(outT.transpose(0, 2, 1))


def kernel(Q, K, V, mask, dk=128):
    from concourse.bass_utils import run_bass_kernel_spmd

    assert int(dk) == 128
    in_maps = host_prep(Q, K, V, mask)
    nc = _get_nc(loop=False)
    res = run_bass_kernel_spmd(nc, in_maps, core_ids=list(range(NCORES)))
    return gather_out(res.results)
